# revision 33
# baseline (speedup 1.0000x reference)
"""Trainium2 Bass kernel for nn_BYOSv1_61211873903141 (scatter_memory).

Math (per batch b):
  q = (H @ W_tok.T) viewed per-head            [T, NH, HD]
  k = prototypes per-head                      [NH, NS, HD]
  score = q.k / sqrt(HD); w = softmax_s(score) [NH, T, NS]
  g = 1 - a*w ; suffix_ex[t] = prod_{t'>t} g[t']
  out[b, s, :] = sum_t a*w[t,s]*suffix_ex[t,s] * h[t]   (per head block)

Device algorithm (per core, token-sliced: core c = (batch c//2, half c%2)):
  - Keff^T[din, (h,s)] = sum_dh W[h*64+dh, din]*proto[s, h*64+dh]/8 is
    precomputed on the host (it only depends on W/prototypes) and shipped as
    fp8 e4m3 weight sets so the score matmul runs in DoubleRow perf mode
    (2 k-chunks per pass, 0.5 cycles/row):
      term1: H8 @ (64*Keff)_hi    term2: H8 @ (64*res)      [fixes K quant]
      term3: (16*H_lo)_8 @ (4*Keff)_8                       [fixes H quant]
    PSUM holds 64*score; the exp activation applies scale=1/64.
  - Layout [(2 heads * 64 slots) partitions, t free]: softmax partition-reduce
    via block-ones matmul; alpha/Z on ACT (Reciprocal, scale=1/alpha);
    broadcast back via selector matmul; u = e*pu on DVE/Pool.
  - suffix products via DVE tensor_tensor_scan (cumprod along free axis) on
    host-time-reversed tokens (prefix in stored order == suffix in true time).
  - einsum2 (contract t) after PE-transposing w_eff back to [t, (h,s)];
    transpose PSUM banks are drained to SBUF by DMA (ACT ring).
  - Host combines the two halves per batch: S = S_late + G_late * S_early.
"""

import numpy as np
import ml_dtypes

B, T, D = 4, 4096, 1024
NH, NS, HD = 16, 64, 64
ALPHA = 0.1
NCORES = 8
TC = T // 2        # tokens per core slice
NJ = NH // 2       # 8 head-pairs (128 partitions = 2 heads x 64 slots)
NQ = 4             # t quarters
QT = TC // NQ      # 512
NKD = D // 128     # 8 din chunks
NTT = TC // 128    # 16 t-tiles of 128
TERMS = 3          # fp8 score terms (2 = faster, 3 = more accurate)
NG3 = 2            # DoubleRow k-groups (of 4) covered by the H-residual term

# packed const block column offsets (bf16, [128, CST_W])
OFF_ID = 0
OFF_ZL = 128                     # 4 x 8   (per-quarter Z selectors)
OFF_SEL = OFF_ZL + NQ * 8        # 4 x 128 (rows 0:8)
CST_W = OFF_SEL + NQ * 128

_BUILT = None
LAST_RESULTS = None


def _build_module():
    import concourse.bacc as bacc
    import concourse.mybir as mybir
    import concourse.tile as tile

    bf16 = mybir.dt.bfloat16
    fp8 = mybir.dt.float8e4
    f32 = mybir.dt.float32
    AF = mybir.ActivationFunctionType
    ALU = mybir.AluOpType
    DR = mybir.MatmulPerfMode.DoubleRow

    nc = bacc.Bacc(None, target_bir_lowering=False)

    khi_d = nc.dram_tensor("khi", [128, NKD, NJ * 128], fp8, kind="ExternalInput")
    klo_d = nc.dram_tensor("klo", [128, NKD, NJ * 128], fp8, kind="ExternalInput")
    ht_d = nc.dram_tensor("ht", [128, NKD, TC], fp8, kind="ExternalInput")
    if TERMS == 3:
        kh4_d = nc.dram_tensor("kh4", [128, 2 * NG3, NJ * 128], fp8, kind="ExternalInput")
        hlo_d = nc.dram_tensor("hlo", [128, 2 * NG3, TC], fp8, kind="ExternalInput")
    hn_d = nc.dram_tensor("hn", [128, NTT, D], bf16, kind="ExternalInput")
    cst_d = nc.dram_tensor("cst", [128, CST_W], bf16, kind="ExternalInput")
    warm_d = nc.dram_tensor("warm", [128, 512], bf16, kind="ExternalInput")
    s_d = nc.dram_tensor("s_out", [NJ, 128, 128], f32, kind="ExternalOutput")
    g_d = nc.dram_tensor("g_out", [128, NJ], f32, kind="ExternalOutput")

    with tile.TileContext(nc) as tc:
        with (
            tc.tile_pool(name="consts", bufs=1) as cpool,
            tc.tile_pool(name="iopool", bufs=1) as iopool,
            tc.tile_pool(name="hnpool", bufs=1) as hnpool,
            tc.tile_pool(name="epool", bufs=3) as epool,
            tc.tile_pool(name="upool", bufs=3) as upool,
            tc.tile_pool(name="rzpool", bufs=2) as rzpool,
            tc.tile_pool(name="work", bufs=2) as work,
            tc.tile_pool(name="wtpool", bufs=3) as wtpool,
            tc.tile_pool(name="ps512", bufs=4, space="PSUM") as ps512,
            tc.tile_pool(name="psz", bufs=1, space="PSUM") as psz,
            tc.tile_pool(name="pst", bufs=2, space="PSUM") as pstp,
            tc.tile_pool(name="pse", bufs=1, space="PSUM") as pse,
        ):
            # ---- input DMAs, ordered so the first score matmuls unblock
            #      as early as possible; K sets split so pair 0's slices
            #      arrive before the tail pairs' ----
            warm = cpool.tile([128, 512], bf16, tag="warm", name="t_warm")
            nc.sync.dma_start(warm[:], warm_d[:])
            khi = iopool.tile([128, NKD, NJ * 128], fp8, tag="khi", name="t_khi")
            klo = iopool.tile([128, NKD, NJ * 128], fp8, tag="klo", name="t_klo")
            ht = iopool.tile([128, NKD, TC], fp8, tag="ht", name="t_ht")
            if TERMS == 3:
                kh4 = iopool.tile([128, 2 * NG3, NJ * 128], fp8, tag="kh4", name="t_kh4")
                hlo = iopool.tile([128, 2 * NG3, TC], fp8, tag="hlo", name="t_hlo")
            nc.sync.dma_start(khi[:, :, 0:128], khi_d[:, :, 0:128])
            nc.sync.dma_start(klo[:, :, 0:128], klo_d[:, :, 0:128])
            if TERMS == 3:
                nc.sync.dma_start(kh4[:, :, 0:128], kh4_d[:, :, 0:128])
            nc.sync.dma_start(ht[:, :, 0:QT], ht_d[:, :, 0:QT])
            if TERMS == 3:
                nc.sync.dma_start(hlo[:, :, 0:QT], hlo_d[:, :, 0:QT])
            nc.sync.dma_start(khi[:, :, 128:512], khi_d[:, :, 128:512])
            nc.sync.dma_start(klo[:, :, 128:512], klo_d[:, :, 128:512])
            if TERMS == 3:
                nc.sync.dma_start(kh4[:, :, 128:512], kh4_d[:, :, 128:512])
            cst = cpool.tile([128, CST_W], bf16, tag="cst", name="t_cst")
            nc.sync.dma_start(cst[:], cst_d[:])
            for q in range(1, NQ):
                nc.sync.dma_start(
                    ht[:, :, QT * q : QT * (q + 1)], ht_d[:, :, QT * q : QT * (q + 1)]
                )
                if TERMS == 3:
                    nc.sync.dma_start(
                        hlo[:, :, QT * q : QT * (q + 1)],
                        hlo_d[:, :, QT * q : QT * (q + 1)],
                    )
            nc.sync.dma_start(khi[:, :, 512:1024], khi_d[:, :, 512:1024])
            nc.sync.dma_start(klo[:, :, 512:1024], klo_d[:, :, 512:1024])
            hn = []
            for kt in range(NTT):
                hnt = hnpool.tile([128, D], bf16, tag=f"hn{kt}", name=f"t_hn{kt}")
                nc.sync.dma_start(hnt[:], hn_d[:, kt, :])
                hn.append(hnt)

            ident = cst[:, OFF_ID : OFF_ID + 128]
            gall = cpool.tile([128, NJ], f32, tag="gall", name="t_gall")

            # warm the PE clock ramp while the input DMAs stream
            psw = ps512.tile([128, QT], f32, tag="sc", bufs=2, name="t_warmps")
            for _ in range(30):
                nc.tensor.matmul(
                    psw[:], warm[:, 0:128], warm[:], start=True, stop=True
                )

            def zl_ap(q):
                o = OFF_ZL + 8 * q
                return cst[:, o : o + 8]

            def sel_ap(q):
                o = OFF_SEL + 128 * q
                return cst[0:8, o : o + 128]

            HT2 = TC // 2
            sets = [(khi, ht, NKD // 2), (klo, ht, NKD // 2)]
            if TERMS == 3 and NG3 > 0:
                sets.append((kh4, hlo))
            NPROD = NKD + NG3 if TERMS == 3 else NKD

            st = {}  # per-pair live state

            def score_mm(j, q):
                """fp8 DoubleRow score matmuls for quarter q -> exp."""
                S = st[j]
                ps = ps512.tile([128, QT], f32, tag="sc", bufs=2, name="t_ps512")
                for c in range(2):
                    t0 = QT * q + 256 * c
                    n = 0
                    for se in sets:
                        ng = se[2] if len(se) > 2 else NG3
                        kt_, dt_ = se[0], se[1]
                        for gi in range(ng):
                            g = 2 * gi
                            n += 1
                            nc.tensor.matmul(
                                ps[:, 256 * c : 256 * (c + 1)],
                                kt_[:, g : g + 2, 128 * j : 128 * (j + 1)],
                                dt_[:, g : g + 2, t0 : t0 + 256],
                                start=(n == 1),
                                stop=(n == NPROD // 2),
                                perf_mode=DR,
                            )
                nc.scalar.activation(
                    S["e"][:, QT * q : QT * (q + 1)], ps[:], AF.Exp, scale=1.0 / 64.0
                )

            def zred(j, q):
                S = st[j]
                nc.tensor.matmul(
                    S["pz"][:],
                    zl_ap(q),
                    S["e"][:, QT * q : QT * (q + 1)],
                    start=(q == 0),
                    stop=(q == NQ - 1),
                )

            def a_begin(j):
                st[j] = {
                    "e": epool.tile([128, TC], bf16, tag="e", name="t_e"),
                    "pz": psz.tile([8, QT], f32, tag="psz", name="t_psz"),
                }

            def a_recip(j):
                """alpha/Z (zl holds 1/alpha, so pz = Z/alpha)."""
                S = st[j]
                rz32 = rzpool.tile([8, QT], f32, tag="rz32", name="t_rz32")
                rz = rzpool.tile([8, QT], bf16, tag="rz", name="t_rz")
                nc.vector.reciprocal_approx_fast(out=rz32[:], in_=S["pz"][:])
                nc.scalar.copy(rz[:], rz32[:])
                S["rz"] = rz

            def a2_half(j, h):
                """broadcast alpha/Z over slots (PE), u = e * pu (DVE/Pool)."""
                S = st[j]
                if h == 0:
                    S["u"] = upool.tile([128, TC], bf16, tag="u", name="t_u")
                u_j = S["u"]
                for q in (2 * h, 2 * h + 1):
                    pu = ps512.tile([128, QT], f32, tag="pu", bufs=2, name="t_psu")
                    nc.tensor.matmul(pu[:], sel_ap(q), S["rz"][:], start=True, stop=True)
                    eng = nc.vector
                    eng.tensor_tensor(
                        out=u_j[:, QT * q : QT * (q + 1)],
                        in0=S["e"][:, QT * q : QT * (q + 1)],
                        in1=pu[:],
                        op=ALU.mult,
                    )

            def g_full(j):
                """g = 1 - u on ACT."""
                S = st[j]
                S["g"] = work.tile([128, TC], f32, tag="g", name="t_g")
                nc.scalar.activation(
                    S["g"][:], S["u"][:], AF.Copy, bias=1.0, scale=-1.0
                )

            def b1_scan_weff(j):
                """suffix scan -> w_eff (DVE), transpose (PE), drain (Pool)."""
                S = st[j]
                g_j = S["g"]
                u_j = S["u"]
                suf = work.tile([128, TC + 1], bf16, tag="suf", name="t_suf")
                w_j = work.tile([128, TC], bf16, tag="weff", name="t_weff")
                wT_j = wtpool.tile([128, TC], bf16, tag="weffT", name="t_weffT")
                nc.vector.memset(suf[:, 0:1], 1.0)
                for h in range(2):
                    lo, hi = h * HT2, (h + 1) * HT2
                    nc.vector.tensor_tensor_scan(
                        out=suf[:, lo + 1 : hi + 1],
                        data0=g_j[:, lo:hi],
                        data1=g_j[:, lo:hi],
                        initial=1.0 if h == 0 else suf[:, lo : lo + 1],
                        op0=ALU.mult,
                        op1=ALU.bypass,
                    )
                    # w_eff = u * suffix_exclusive (SBUF-only, so Pool can run it)
                    nc.gpsimd.tensor_tensor(
                        out=w_j[:, lo:hi],
                        in0=u_j[:, lo:hi],
                        in1=suf[:, lo:hi],
                        op=ALU.mult,
                    )
                # transpose -> [t, hs]; gr=1 drained to SBUF by Pool here,
                # gr=0 drained by ACT later (b1_drain_act) to keep exp ahead
                S["pt0"] = None
                for gr in range(2):
                    pt = pstp.tile([128, 1024], bf16, tag="pstp", name="t_pstp")
                    for bb in range(8):
                        kt = 8 * gr + bb
                        nc.tensor.transpose(
                            pt[:, 128 * bb : 128 * (bb + 1)],
                            w_j[:, 128 * kt : 128 * (kt + 1)],
                            ident,
                        )
                    if gr == 0:
                        S["pt0"] = pt
                    else:
                        nc.vector.tensor_copy(
                            out=wT_j[:, 1024 * gr : 1024 * (gr + 1)], in_=pt[:]
                        )
                nc.vector.tensor_copy(out=gall[:, j : j + 1], in_=suf[:, TC : TC + 1])
                S["wT"] = wT_j

            def b1_drain_act(j):
                S = st[j]
                nc.scalar.copy(S["wT"][:, 0:1024], S["pt0"][:])

            def b2_einsum(j):
                S = st[j]
                wT_j = S["wT"]
                pS = pse.tile([128, 128], f32, tag="pse", name="t_pse")
                for kt in range(NTT):
                    nc.tensor.matmul(
                        pS[:],
                        wT_j[:, 128 * kt : 128 * (kt + 1)],
                        hn[kt][:, 128 * j : 128 * (j + 1)],
                        start=(kt == 0),
                        stop=(kt == NTT - 1),
                    )
                s_sb = wtpool.tile([128, 128], f32, tag="ssb", name="t_ssb")
                nc.scalar.copy(s_sb[:], pS[:])
                nc.sync.dma_start(s_d[j], s_sb[:])
                del st[j]

            # Software pipeline, issue-ordered so no engine queue head blocks
            # on work that is not yet ready.
            for s in range(NJ + 2):
                if s < NJ:
                    a_begin(s)
                    score_mm(s, 0)
                    score_mm(s, 1)
                    zred(s, 0)
                if 1 <= s <= NJ:
                    a2_half(s - 1, 0)
                if s < NJ:
                    score_mm(s, 2)
                    zred(s, 1)
                if 1 <= s <= NJ:
                    a2_half(s - 1, 1)
                if s < NJ:
                    score_mm(s, 3)
                    zred(s, 2)
                if 2 <= s <= NJ + 1:
                    b1_scan_weff(s - 2)
                if 4 <= s <= NJ:
                    b2_einsum(s - 4)
                if s < NJ:
                    zred(s, 3)
                if 1 <= s <= NJ:
                    g_full(s - 1)
                if s < NJ:
                    a_recip(s)
                if 2 <= s <= NJ + 1:
                    b1_drain_act(s - 2)
                if s == NJ + 1:
                    for jj in range(NJ - 3, NJ):
                        b2_einsum(jj)

            nc.sync.dma_start(g_d[:], gall[:])

    nc.compile()
    return nc


def _host_consts():
    cst = np.zeros((128, CST_W), np.float32)
    cst[:, OFF_ID : OFF_ID + 128] = np.eye(128, dtype=np.float32)
    for q in range(NQ):
        for l in range(2):
            o = OFF_ZL + 8 * q
            cst[l * 64 : (l + 1) * 64, o + 2 * q + l] = 1.0 / ALPHA
            o = OFF_SEL + 128 * q
            cst[2 * q + l, o + l * 64 : o + (l + 1) * 64] = 1.0
    return cst


def _chunk128(a):
    """[D, X] -> [128, D//128, X] with d = 128*k + p."""
    Dx, X = a.shape
    return np.ascontiguousarray(a.reshape(Dx // 128, 128, X).transpose(1, 0, 2))


def kernel(H, prototypes, W_tok):
    global _BUILT, LAST_RESULTS
    from concourse.bass_utils import run_bass_kernel_spmd

    if _BUILT is None:
        _BUILT = _build_module()
    nc = _BUILT

    bf = ml_dtypes.bfloat16
    e4 = ml_dtypes.float8_e4m3
    H = np.asarray(H, np.float32)
    prototypes = np.asarray(prototypes, np.float32)
    W_tok = np.asarray(W_tok, np.float32)

    # Keff[(h,s), din] = sum_dh proto[s, h*64+dh] * W[h*64+dh, din] / 8
    P3 = prototypes.reshape(NS, NH, HD)
    W3 = W_tok.reshape(NH, HD, D)
    Keff = (np.einsum("shd,hdn->hsn", P3, W3) / 8.0).reshape(NH * NS, D)

    K_hi = (Keff * 64.0).astype(e4)
    K_res = Keff - K_hi.astype(np.float32) / 64.0
    K_lo = (K_res * 64.0).astype(e4)
    khi_dev = _chunk128(K_hi.T.astype(np.float32)).astype(e4)
    klo_dev = _chunk128(K_lo.T.astype(np.float32)).astype(e4)
    if TERMS == 3:
        kh4_dev = _chunk128((Keff * 4.0).T[: 256 * NG3]).astype(e4)

    cst = _host_consts().astype(bf)

    in_maps = []
    for c in range(NCORES):
        b, th = c // 2, c % 2
        Hs = np.ascontiguousarray(H[b, th * TC : (th + 1) * TC, :][::-1])  # [TC, D]
        H8 = Hs.astype(e4)
        m = {
            "khi": khi_dev,
            "klo": klo_dev,
            "ht": _chunk128(np.ascontiguousarray(H8.astype(np.float32).T)).astype(e4),
            "hn": np.ascontiguousarray(
                Hs.reshape(NTT, 128, D).transpose(1, 0, 2)
            ).astype(bf),
            "cst": cst,
            "warm": np.full((128, 512), 0.25, bf),
        }
        if TERMS == 3:
            Hlo = (Hs - H8.astype(np.float32)) * 16.0
            m["kh4"] = kh4_dev
            m["hlo"] = _chunk128(np.ascontiguousarray(Hlo.T[: 256 * NG3])).astype(e4)
        in_maps.append(m)

    res = run_bass_kernel_spmd(nc, in_maps, core_ids=list(range(NCORES)))
    LAST_RESULTS = res

    percore = []
    for c in range(NCORES):
        r = res.results[c]
        s_blocks = r["s_out"]  # [NJ, 128, 128]
        g_cols = r["g_out"]  # [128, NJ]
        S_c = np.zeros((NH, NS, HD), np.float32)
        G_c = np.zeros((NH, NS), np.float32)
        for j in range(NJ):
            for l in range(2):
                h = 2 * j + l
                S_c[h] = s_blocks[j, l * 64 : (l + 1) * 64, l * 64 : (l + 1) * 64]
                G_c[h] = g_cols[l * 64 : (l + 1) * 64, j]
        percore.append((S_c, G_c))

    out = np.zeros((B, NS, D), np.float32)
    for b in range(B):
        S_e, G_e = percore[2 * b]      # tokens [0, TC)
        S_l, G_l = percore[2 * b + 1]  # tokens [TC, T)
        S_b = S_l + G_l[:, :, None] * S_e
        out[b] = S_b.transpose(1, 0, 2).reshape(NS, D)
    return out


# revision 44
# speedup vs baseline: 1.0899x; 1.0899x over previous
"""Trainium2 Bass kernel for nn_BYOSv1_61211873903141 (scatter_memory).

Math (per batch b):
  q = (H @ W_tok.T) viewed per-head            [T, NH, HD]
  k = prototypes per-head                      [NH, NS, HD]
  score = q.k / sqrt(HD); w = softmax_s(score) [NH, T, NS]
  g = 1 - a*w ; suffix_ex[t] = prod_{t'>t} g[t']
  out[b, s, :] = sum_t a*w[t,s]*suffix_ex[t,s] * h[t]   (per head block)

Device algorithm (per core, token-sliced: core c = (batch c//2, half c%2)):
  - Keff^T[din, (h,s)] = sum_dh W[h*64+dh, din]*proto[s, h*64+dh]/8 is
    precomputed on the host (it only depends on W/prototypes) and shipped as
    fp8 e4m3 weight sets so the score matmul runs in DoubleRow perf mode
    (2 k-chunks per pass, 0.5 cycles/row):
      term1: H8 @ (64*Keff)_hi    term2: H8 @ (64*res)      [fixes K quant]
      term3: (16*H_lo)_8 @ (4*Keff)_8                       [fixes H quant]
    PSUM holds 64*score; the exp activation applies scale=1/64.
  - Layout [(2 heads * 64 slots) partitions, t free]: softmax partition-reduce
    via block-ones matmul; alpha/Z on ACT (Reciprocal, scale=1/alpha);
    broadcast back via selector matmul; u = e*pu on DVE/Pool.
  - suffix products via DVE tensor_tensor_scan (cumprod along free axis) on
    host-time-reversed tokens (prefix in stored order == suffix in true time).
  - einsum2 (contract t) after PE-transposing w_eff back to [t, (h,s)];
    transpose PSUM banks are drained to SBUF by DMA (ACT ring).
  - Host combines the two halves per batch: S = S_late + G_late * S_early.
"""

import numpy as np
import ml_dtypes

B, T, D = 4, 4096, 1024
NH, NS, HD = 16, 64, 64
ALPHA = 0.1
NCORES = 8
TC = T // 2        # tokens per core slice
NJ = NH // 2       # 8 head-pairs (128 partitions = 2 heads x 64 slots)
NQ = 4             # t quarters
QT = TC // NQ      # 512
NKD = D // 128     # 8 din chunks
NTT = TC // 128    # 16 t-tiles of 128
TERMS = 3          # fp8 score terms (2 = faster, 3 = more accurate)
RZ_DMA = False     # broadcast alpha/Z via DRAM bounce (else PE matmuls)
NG3 = 2            # DoubleRow k-groups (of 4) covered by the H-residual term

# packed const block column offsets (bf16, [128, CST_W])
OFF_ID = 0
OFF_ZL = 128                     # 4 x 8   (per-quarter Z selectors)
OFF_SEL = OFF_ZL + NQ * 8        # 4 x 128 (rows 0:8)
CST_W = OFF_SEL + NQ * 128

_BUILT = None
LAST_RESULTS = None


def _build_module():
    import concourse.bacc as bacc
    import concourse.mybir as mybir
    import concourse.tile as tile

    bf16 = mybir.dt.bfloat16
    fp8 = mybir.dt.float8e4
    f32 = mybir.dt.float32
    AF = mybir.ActivationFunctionType
    ALU = mybir.AluOpType
    DR = mybir.MatmulPerfMode.DoubleRow

    nc = bacc.Bacc(None, target_bir_lowering=False)

    khi_d = nc.dram_tensor("khi", [128, NKD, NJ * 128], fp8, kind="ExternalInput")
    klo_d = nc.dram_tensor("klo", [128, NKD, NJ * 128], fp8, kind="ExternalInput")
    ht_d = nc.dram_tensor("ht", [128, NKD, TC], fp8, kind="ExternalInput")
    if TERMS == 3:
        kh4_d = nc.dram_tensor("kh4", [128, 2 * NG3, NJ * 128], fp8, kind="ExternalInput")
        hlo_d = nc.dram_tensor("hlo", [128, 2 * NG3, TC], fp8, kind="ExternalInput")
    hn_d = nc.dram_tensor("hn", [128, NTT, D], bf16, kind="ExternalInput")
    cst_d = nc.dram_tensor("cst", [128, CST_W], bf16, kind="ExternalInput")
    warm_d = nc.dram_tensor("warm", [128, 512], bf16, kind="ExternalInput")
    rz8_d = [
        nc.dram_tensor(f"rz8_{jj}", [2, NQ * QT], bf16, kind="Internal")
        for jj in range(NJ)
    ]
    s_d = nc.dram_tensor("s_out", [NJ, 128, 128], f32, kind="ExternalOutput")
    g_d = nc.dram_tensor("g_out", [128, NJ], f32, kind="ExternalOutput")

    with tile.TileContext(nc) as tc:
        with (
            tc.tile_pool(name="consts", bufs=1) as cpool,
            tc.tile_pool(name="iopool", bufs=1) as iopool,
            tc.tile_pool(name="hnpool", bufs=1) as hnpool,
            tc.tile_pool(name="epool", bufs=3) as epool,
            tc.tile_pool(name="upool", bufs=3) as upool,
            tc.tile_pool(name="rzpool", bufs=2) as rzpool,
            tc.tile_pool(name="work", bufs=2) as work,
            tc.tile_pool(name="wtpool", bufs=3) as wtpool,
            tc.tile_pool(name="ps512", bufs=4, space="PSUM") as ps512,
            tc.tile_pool(name="psz", bufs=1, space="PSUM") as psz,
            tc.tile_pool(name="pst", bufs=2, space="PSUM") as pstp,
            tc.tile_pool(name="pse", bufs=1, space="PSUM") as pse,
        ):
            # ---- input DMAs, ordered so the first score matmuls unblock
            #      as early as possible; K sets split so pair 0's slices
            #      arrive before the tail pairs' ----
            warm = cpool.tile([128, 512], bf16, tag="warm", name="t_warm")
            nc.sync.dma_start(warm[:], warm_d[:])
            khi = iopool.tile([128, NKD, NJ * 128], fp8, tag="khi", name="t_khi")
            klo = iopool.tile([128, NKD, NJ * 128], fp8, tag="klo", name="t_klo")
            ht = iopool.tile([128, NKD, TC], fp8, tag="ht", name="t_ht")
            if TERMS == 3:
                kh4 = iopool.tile([128, 2 * NG3, NJ * 128], fp8, tag="kh4", name="t_kh4")
                hlo = iopool.tile([128, 2 * NG3, TC], fp8, tag="hlo", name="t_hlo")
            nc.sync.dma_start(khi[:, :, 0:128], khi_d[:, :, 0:128])
            nc.sync.dma_start(klo[:, :, 0:128], klo_d[:, :, 0:128])
            if TERMS == 3:
                nc.sync.dma_start(kh4[:, :, 0:128], kh4_d[:, :, 0:128])
            nc.sync.dma_start(ht[:, :, 0:QT], ht_d[:, :, 0:QT])
            if TERMS == 3:
                nc.sync.dma_start(hlo[:, :, 0:QT], hlo_d[:, :, 0:QT])
            cst = cpool.tile([128, CST_W], bf16, tag="cst", name="t_cst")
            nc.sync.dma_start(cst[:], cst_d[:])
            for q in range(1, NQ):
                nc.sync.dma_start(
                    ht[:, :, QT * q : QT * (q + 1)], ht_d[:, :, QT * q : QT * (q + 1)]
                )
                if TERMS == 3:
                    nc.sync.dma_start(
                        hlo[:, :, QT * q : QT * (q + 1)],
                        hlo_d[:, :, QT * q : QT * (q + 1)],
                    )
            nc.sync.dma_start(khi[:, :, 128:512], khi_d[:, :, 128:512])
            nc.sync.dma_start(klo[:, :, 128:512], klo_d[:, :, 128:512])
            if TERMS == 3:
                nc.sync.dma_start(kh4[:, :, 128:512], kh4_d[:, :, 128:512])
            nc.sync.dma_start(khi[:, :, 512:1024], khi_d[:, :, 512:1024])
            nc.sync.dma_start(klo[:, :, 512:1024], klo_d[:, :, 512:1024])
            if TERMS == 3:
                nc.sync.dma_start(kh4[:, :, 512:1024], kh4_d[:, :, 512:1024])
            hn = [
                hnpool.tile([128, D], bf16, tag=f"hn{kt}", name=f"t_hn{kt}")
                for kt in range(NTT)
            ]

            ident = cst[:, OFF_ID : OFF_ID + 128]
            gall = cpool.tile([128, NJ], f32, tag="gall", name="t_gall")

            # warm the PE clock ramp while the input DMAs stream
            psw = ps512.tile([128, QT], f32, tag="sc", bufs=4 if RZ_DMA else 2, name="t_warmps")
            for _ in range(12):
                nc.tensor.matmul(
                    psw[:], warm[:, 0:128], warm[:], start=True, stop=True
                )

            def zl_ap(q):
                o = OFF_ZL + 8 * q
                return cst[:, o : o + 8]

            def sel_ap(q):
                o = OFF_SEL + 128 * q
                return cst[0:8, o : o + 128]

            HT2 = TC // 2
            sets = [(khi, ht, NKD // 2), (klo, ht, NKD // 2)]
            if TERMS == 3 and NG3 > 0:
                sets.append((kh4, hlo))
            NPROD = NKD + NG3 if TERMS == 3 else NKD

            st = {}  # per-pair live state

            def score_mm(j, q):
                """fp8 DoubleRow score matmuls for quarter q -> exp."""
                S = st[j]
                ps = ps512.tile([128, QT], f32, tag="sc", bufs=4 if RZ_DMA else 2, name="t_ps512")
                for c in range(2):
                    t0 = QT * q + 256 * c
                    n = 0
                    for se in sets:
                        ng = se[2] if len(se) > 2 else NG3
                        kt_, dt_ = se[0], se[1]
                        for gi in range(ng):
                            g = 2 * gi
                            n += 1
                            nc.tensor.matmul(
                                ps[:, 256 * c : 256 * (c + 1)],
                                kt_[:, g : g + 2, 128 * j : 128 * (j + 1)],
                                dt_[:, g : g + 2, t0 : t0 + 256],
                                start=(n == 1),
                                stop=(n == NPROD),
                                perf_mode=DR,
                            )
                nc.scalar.activation(
                    S["e"][:, QT * q : QT * (q + 1)], ps[:], AF.Exp, scale=1.0 / 64.0
                )

            def zred(j, q):
                S = st[j]
                nc.tensor.matmul(
                    S["pz"][:],
                    zl_ap(q),
                    S["e"][:, QT * q : QT * (q + 1)],
                    start=(q == 0),
                    stop=(q == NQ - 1),
                )

            def a_begin(j):
                st[j] = {
                    "e": epool.tile([128, TC], bf16, tag="e", name="t_e"),
                    "pz": psz.tile([8, QT], f32, tag="psz", name="t_psz"),
                }

            def a_recip(j):
                """alpha/Z (zl holds 1/alpha, so pz = Z/alpha)."""
                S = st[j]
                rz32 = rzpool.tile([8, QT], f32, tag="rz32", name="t_rz32")
                rz = rzpool.tile([8, QT], bf16, tag="rz", name="t_rz")
                nc.vector.reciprocal_approx_fast(out=rz32[:], in_=S["pz"][:])
                nc.scalar.copy(rz[:], rz32[:])
                S["rz"] = rz
                if RZ_DMA:
                    # bounce alpha/Z through DRAM to broadcast across
                    # partitions: row 2q+l of rz -> slot-partitions of head l
                    nc.sync.dma_start(
                        rz8_d[j][:].rearrange("l (q t) -> l q t", q=NQ),
                        rz[:].rearrange("(q l) t -> l q t", l=2),
                    )
                    rzb = rzpool.tile([128, TC], bf16, tag="rzb", name="t_rzb")
                    nc.sync.dma_start(
                        rzb[:].rearrange("(l s) t -> s l t", l=2),
                        rz8_d[j][:].partition_broadcast(64),
                    )
                    S["rzb"] = rzb

            def u_q(j, q):
                """u = e * (alpha/Z broadcast), one quarter (pu matmul + DVE)."""
                S = st[j]
                if q == 0:
                    S["u"] = upool.tile([128, TC], bf16, tag="u", name="t_u")
                if RZ_DMA:
                    if q == 0:
                        nc.vector.tensor_tensor(
                            out=S["u"][:], in0=S["e"][:], in1=S["rzb"][:],
                            op=ALU.mult,
                        )
                    return
                pu = ps512.tile([128, QT], f32, tag="pu", bufs=2, name="t_psu")
                nc.tensor.matmul(pu[:], sel_ap(q), S["rz"][:], start=True, stop=True)
                nc.vector.tensor_tensor(
                    out=S["u"][:, QT * q : QT * (q + 1)],
                    in0=S["e"][:, QT * q : QT * (q + 1)],
                    in1=pu[:],
                    op=ALU.mult,
                )

            def g_h(j, h):
                """g = 1 - u on Pool (SBUF-only), one half."""
                S = st[j]
                if h == 0:
                    S["g"] = work.tile([128, TC], f32, tag="g", name="t_g")
                lo, hi = h * HT2, (h + 1) * HT2
                nc.gpsimd.tensor_scalar(
                    out=S["g"][:, lo:hi],
                    in0=S["u"][:, lo:hi],
                    scalar1=-1.0,
                    scalar2=1.0,
                    op0=ALU.mult,
                    op1=ALU.add,
                )

            def scan_h(j, h):
                """suffix scan half (DVE)."""
                S = st[j]
                if h == 0:
                    S["suf"] = work.tile([128, TC + 1], bf16, tag="suf", name="t_suf")
                    nc.vector.memset(S["suf"][:, 0:1], 1.0)
                suf = S["suf"]
                lo, hi = h * HT2, (h + 1) * HT2
                nc.vector.tensor_tensor_scan(
                    out=suf[:, lo + 1 : hi + 1],
                    data0=S["g"][:, lo:hi],
                    data1=S["g"][:, lo:hi],
                    initial=1.0 if h == 0 else suf[:, lo : lo + 1],
                    op0=ALU.mult,
                    op1=ALU.bypass,
                )

            def weff_h(j, h):
                """w_eff = u * suffix_exclusive, one half (DVE)."""
                S = st[j]
                if h == 0:
                    S["w"] = work.tile([128, TC], bf16, tag="weff", name="t_weff")
                lo, hi = h * HT2, (h + 1) * HT2
                nc.vector.tensor_tensor(
                    out=S["w"][:, lo:hi],
                    in0=S["u"][:, lo:hi],
                    in1=S["suf"][:, lo:hi],
                    op=ALU.mult,
                )

            def b1_tp(j):
                """transpose w_eff -> [t, hs] (PE); gr=0 drained later by ACT."""
                S = st[j]
                w_j = S["w"]
                wT_j = wtpool.tile([128, TC], bf16, tag="weffT", name="t_weffT")
                S["pt0"] = None
                for gr in range(2):
                    pt = pstp.tile([128, 1024], bf16, tag="pstp", name="t_pstp")
                    for bb in range(8):
                        kt = 8 * gr + bb
                        nc.tensor.transpose(
                            pt[:, 128 * bb : 128 * (bb + 1)],
                            w_j[:, 128 * kt : 128 * (kt + 1)],
                            ident,
                        )
                    if gr == 0:
                        S["pt0"] = pt
                    else:
                        nc.scalar.copy(wT_j[:, 1024 * gr : 1024 * (gr + 1)], pt[:])
                nc.vector.tensor_copy(
                    out=gall[:, j : j + 1], in_=S["suf"][:, TC : TC + 1]
                )
                S["wT"] = wT_j

            def b1_drain_act(j):
                S = st[j]
                nc.scalar.copy(S["wT"][:, 0:1024], S["pt0"][:])

            def b2_einsum(j):
                S = st[j]
                wT_j = S["wT"]
                pS = pse.tile([128, 128], f32, tag="pse", name="t_pse")
                for kt in range(NTT):
                    nc.tensor.matmul(
                        pS[:],
                        wT_j[:, 128 * kt : 128 * (kt + 1)],
                        hn[kt][:, 128 * j : 128 * (j + 1)],
                        start=(kt == 0),
                        stop=(kt == NTT - 1),
                    )
                s_sb = wtpool.tile([128, 128], f32, tag="ssb", name="t_ssb")
                nc.scalar.copy(s_sb[:], pS[:])
                nc.sync.dma_start(s_d[j], s_sb[:])
                del st[j]

            # Software pipeline, issue-ordered so no engine queue head blocks
            # on work that is not yet ready; the u->g->scan->w_eff chain of
            # consecutive pairs is interleaved at quarter/half granularity.
            for s in range(NJ + 2):
                A = s < NJ          # score pair s
                U = 1 <= s <= NJ    # u/g pair s-1
                B = 2 <= s <= NJ + 1  # scan/weff/tp pair s-2
                if A:
                    a_begin(s)
                    score_mm(s, 0)
                    score_mm(s, 1)
                    zred(s, 0)
                if U:
                    u_q(s - 1, 0)
                if B:
                    scan_h(s - 2, 0)
                if U:
                    u_q(s - 1, 1)
                if B:
                    weff_h(s - 2, 0)
                if U:
                    g_h(s - 1, 0)
                if A:
                    score_mm(s, 2)
                    zred(s, 1)
                if U:
                    u_q(s - 1, 2)
                if B:
                    scan_h(s - 2, 1)
                if U:
                    u_q(s - 1, 3)
                if B:
                    weff_h(s - 2, 1)
                if U:
                    g_h(s - 1, 1)
                if A:
                    score_mm(s, 3)
                    zred(s, 2)
                if B:
                    b1_tp(s - 2)
                if 4 <= s <= NJ:
                    b2_einsum(s - 4)
                if A:
                    zred(s, 3)
                    a_recip(s)
                if B:
                    b1_drain_act(s - 2)
                if s < NQ:
                    for kt in range(4 * s, 4 * s + 4):
                        nc.sync.dma_start(hn[kt][:], hn_d[:, kt, :])
                if s == NJ + 1:
                    for jj in range(NJ - 3, NJ):
                        b2_einsum(jj)

            nc.sync.dma_start(g_d[:], gall[:])

    nc.compile()
    return nc


def _host_consts():
    cst = np.zeros((128, CST_W), np.float32)
    cst[:, OFF_ID : OFF_ID + 128] = np.eye(128, dtype=np.float32)
    for q in range(NQ):
        for l in range(2):
            o = OFF_ZL + 8 * q
            cst[l * 64 : (l + 1) * 64, o + 2 * q + l] = 1.0 / ALPHA
            o = OFF_SEL + 128 * q
            cst[2 * q + l, o + l * 64 : o + (l + 1) * 64] = 1.0
    return cst


def _chunk128(a):
    """[D, X] -> [128, D//128, X] with d = 128*k + p."""
    Dx, X = a.shape
    return np.ascontiguousarray(a.reshape(Dx // 128, 128, X).transpose(1, 0, 2))


def kernel(H, prototypes, W_tok):
    global _BUILT, LAST_RESULTS
    from concourse.bass_utils import run_bass_kernel_spmd

    if _BUILT is None:
        _BUILT = _build_module()
    nc = _BUILT

    bf = ml_dtypes.bfloat16
    e4 = ml_dtypes.float8_e4m3
    H = np.asarray(H, np.float32)
    prototypes = np.asarray(prototypes, np.float32)
    W_tok = np.asarray(W_tok, np.float32)

    # Keff[(h,s), din] = sum_dh proto[s, h*64+dh] * W[h*64+dh, din] / 8
    P3 = prototypes.reshape(NS, NH, HD)
    W3 = W_tok.reshape(NH, HD, D)
    Keff = (np.einsum("shd,hdn->hsn", P3, W3) / 8.0).reshape(NH * NS, D)

    K_hi = (Keff * 64.0).astype(e4)
    K_res = Keff - K_hi.astype(np.float32) / 64.0
    K_lo = (K_res * 64.0).astype(e4)
    khi_dev = _chunk128(K_hi.T.astype(np.float32)).astype(e4)
    klo_dev = _chunk128(K_lo.T.astype(np.float32)).astype(e4)
    if TERMS == 3:
        kh4_dev = _chunk128((Keff * 4.0).T[: 256 * NG3]).astype(e4)

    cst = _host_consts().astype(bf)

    in_maps = []
    for c in range(NCORES):
        b, th = c // 2, c % 2
        Hs = np.ascontiguousarray(H[b, th * TC : (th + 1) * TC, :][::-1])  # [TC, D]
        H8 = Hs.astype(e4)
        m = {
            "khi": khi_dev,
            "klo": klo_dev,
            "ht": _chunk128(np.ascontiguousarray(H8.astype(np.float32).T)).astype(e4),
            "hn": np.ascontiguousarray(
                Hs.reshape(NTT, 128, D).transpose(1, 0, 2)
            ).astype(bf),
            "cst": cst,
            "warm": np.full((128, 512), 0.25, bf),
        }
        if TERMS == 3:
            Hlo = (Hs - H8.astype(np.float32)) * 16.0
            m["kh4"] = kh4_dev
            m["hlo"] = _chunk128(np.ascontiguousarray(Hlo.T[: 256 * NG3])).astype(e4)
        in_maps.append(m)

    res = run_bass_kernel_spmd(nc, in_maps, core_ids=list(range(NCORES)))
    LAST_RESULTS = res

    percore = []
    for c in range(NCORES):
        r = res.results[c]
        s_blocks = r["s_out"]  # [NJ, 128, 128]
        g_cols = r["g_out"]  # [128, NJ]
        S_c = np.zeros((NH, NS, HD), np.float32)
        G_c = np.zeros((NH, NS), np.float32)
        for j in range(NJ):
            for l in range(2):
                h = 2 * j + l
                S_c[h] = s_blocks[j, l * 64 : (l + 1) * 64, l * 64 : (l + 1) * 64]
                G_c[h] = g_cols[l * 64 : (l + 1) * 64, j]
        percore.append((S_c, G_c))

    out = np.zeros((B, NS, D), np.float32)
    for b in range(B):
        S_e, G_e = percore[2 * b]      # tokens [0, TC)
        S_l, G_l = percore[2 * b + 1]  # tokens [TC, T)
        S_b = S_l + G_l[:, :, None] * S_e
        out[b] = S_b.transpose(1, 0, 2).reshape(NS, D)
    return out


# revision 50
# speedup vs baseline: 1.0947x; 1.0044x over previous
"""Trainium2 Bass kernel for nn_BYOSv1_61211873903141 (scatter_memory).

Math (per batch b):
  q = (H @ W_tok.T) viewed per-head            [T, NH, HD]
  k = prototypes per-head                      [NH, NS, HD]
  score = q.k / sqrt(HD); w = softmax_s(score) [NH, T, NS]
  g = 1 - a*w ; suffix_ex[t] = prod_{t'>t} g[t']
  out[b, s, :] = sum_t a*w[t,s]*suffix_ex[t,s] * h[t]   (per head block)

Device algorithm (per core, token-sliced: core c = (batch c//2, half c%2)):
  - Keff^T[din, (h,s)] = sum_dh W[h*64+dh, din]*proto[s, h*64+dh]/8 is
    precomputed on the host (it only depends on W/prototypes) and shipped as
    fp8 e4m3 weight sets so the score matmul runs in DoubleRow perf mode
    (2 k-chunks per pass, 0.5 cycles/row):
      term1: H8 @ (64*Keff)_hi    term2: H8 @ (64*res)      [fixes K quant]
      term3: (16*H_lo)_8 @ (4*Keff)_8                       [fixes H quant]
    PSUM holds 64*score; the exp activation applies scale=1/64.
  - Layout [(2 heads * 64 slots) partitions, t free]: softmax partition-reduce
    via block-ones matmul; alpha/Z on ACT (Reciprocal, scale=1/alpha);
    broadcast back via selector matmul; u = e*pu on DVE/Pool.
  - suffix products via DVE tensor_tensor_scan (cumprod along free axis) on
    host-time-reversed tokens (prefix in stored order == suffix in true time).
  - einsum2 (contract t) after PE-transposing w_eff back to [t, (h,s)];
    transpose PSUM banks are drained to SBUF by DMA (ACT ring).
  - Host combines the two halves per batch: S = S_late + G_late * S_early.
"""

import numpy as np
import ml_dtypes

B, T, D = 4, 4096, 1024
NH, NS, HD = 16, 64, 64
ALPHA = 0.1
NCORES = 8
TC = T // 2        # tokens per core slice
NJ = NH // 2       # 8 head-pairs (128 partitions = 2 heads x 64 slots)
NQ = 4             # t quarters
QT = TC // NQ      # 512
NKD = D // 128     # 8 din chunks
NTT = TC // 128    # 16 t-tiles of 128
TERMS = 3          # fp8 score terms (2 = faster, 3 = more accurate)
RZ_DMA = False     # broadcast alpha/Z via DRAM bounce (else PE matmuls)
NG3 = 2            # DoubleRow k-groups (of 4) covered by the H-residual term

# packed const block column offsets (bf16, [128, CST_W])
OFF_ID = 0
OFF_ZL = 128                     # 4 x 8   (per-quarter Z selectors)
OFF_SEL = OFF_ZL + NQ * 8        # 4 x 128 (rows 0:8)
CST_W = OFF_SEL + NQ * 128

_BUILT = None
LAST_RESULTS = None


def _build_module():
    import concourse.bacc as bacc
    import concourse.mybir as mybir
    import concourse.tile as tile

    bf16 = mybir.dt.bfloat16
    fp8 = mybir.dt.float8e4
    f32 = mybir.dt.float32
    AF = mybir.ActivationFunctionType
    ALU = mybir.AluOpType
    DR = mybir.MatmulPerfMode.DoubleRow

    nc = bacc.Bacc(None, target_bir_lowering=False)

    khi_d = nc.dram_tensor("khi", [128, NKD, NJ * 128], fp8, kind="ExternalInput")
    klo_d = nc.dram_tensor("klo", [128, NKD, NJ * 128], fp8, kind="ExternalInput")
    ht_d = nc.dram_tensor("ht", [128, NKD, TC], fp8, kind="ExternalInput")
    if TERMS == 3:
        kh4_d = nc.dram_tensor("kh4", [128, 2 * NG3, NJ * 128], fp8, kind="ExternalInput")
        hlo_d = nc.dram_tensor("hlo", [128, 2 * NG3, TC], fp8, kind="ExternalInput")
    hn_d = nc.dram_tensor("hn", [128, NTT, D], bf16, kind="ExternalInput")
    cst_d = nc.dram_tensor("cst", [128, CST_W], bf16, kind="ExternalInput")
    warm_d = nc.dram_tensor("warm", [128, 512], bf16, kind="ExternalInput")
    rz8_d = [
        nc.dram_tensor(f"rz8_{jj}", [2, NQ * QT], bf16, kind="Internal")
        for jj in range(NJ)
    ]
    s_d = nc.dram_tensor("s_out", [NJ, 128, 128], f32, kind="ExternalOutput")
    g_d = nc.dram_tensor("g_out", [128, NJ], f32, kind="ExternalOutput")

    with tile.TileContext(nc) as tc:
        with (
            tc.tile_pool(name="consts", bufs=1) as cpool,
            tc.tile_pool(name="iopool", bufs=1) as iopool,
            tc.tile_pool(name="hnpool", bufs=1) as hnpool,
            tc.tile_pool(name="epool", bufs=4) as epool,
            tc.tile_pool(name="upool", bufs=4) as upool,
            tc.tile_pool(name="rzpool", bufs=2) as rzpool,
            tc.tile_pool(name="work", bufs=3) as work,
            tc.tile_pool(name="wtpool", bufs=4) as wtpool,
            tc.tile_pool(name="ps512", bufs=4, space="PSUM") as ps512,
            tc.tile_pool(name="psz", bufs=1, space="PSUM") as psz,
            tc.tile_pool(name="pst", bufs=2, space="PSUM") as pstp,
            tc.tile_pool(name="pse", bufs=1, space="PSUM") as pse,
        ):
            # ---- input DMAs, ordered so the first score matmuls unblock
            #      as early as possible; K sets split so pair 0's slices
            #      arrive before the tail pairs' ----
            warm = cpool.tile([128, 512], bf16, tag="warm", name="t_warm")
            nc.sync.dma_start(warm[:], warm_d[:])
            khi = iopool.tile([128, NKD, NJ * 128], fp8, tag="khi", name="t_khi")
            klo = iopool.tile([128, NKD, NJ * 128], fp8, tag="klo", name="t_klo")
            ht = iopool.tile([128, NKD, TC], fp8, tag="ht", name="t_ht")
            if TERMS == 3:
                kh4 = iopool.tile([128, 2 * NG3, NJ * 128], fp8, tag="kh4", name="t_kh4")
                hlo = iopool.tile([128, 2 * NG3, TC], fp8, tag="hlo", name="t_hlo")
            nc.sync.dma_start(khi[:, :, 0:128], khi_d[:, :, 0:128])
            nc.sync.dma_start(klo[:, :, 0:128], klo_d[:, :, 0:128])
            if TERMS == 3:
                nc.sync.dma_start(kh4[:, :, 0:128], kh4_d[:, :, 0:128])
            nc.sync.dma_start(ht[:, :, 0:QT], ht_d[:, :, 0:QT])
            if TERMS == 3:
                nc.sync.dma_start(hlo[:, :, 0:QT], hlo_d[:, :, 0:QT])
            cst = cpool.tile([128, CST_W], bf16, tag="cst", name="t_cst")
            nc.sync.dma_start(cst[:], cst_d[:])
            for q in range(1, NQ):
                nc.sync.dma_start(
                    ht[:, :, QT * q : QT * (q + 1)], ht_d[:, :, QT * q : QT * (q + 1)]
                )
                if TERMS == 3:
                    nc.sync.dma_start(
                        hlo[:, :, QT * q : QT * (q + 1)],
                        hlo_d[:, :, QT * q : QT * (q + 1)],
                    )
            nc.sync.dma_start(khi[:, :, 128:512], khi_d[:, :, 128:512])
            nc.sync.dma_start(klo[:, :, 128:512], klo_d[:, :, 128:512])
            if TERMS == 3:
                nc.sync.dma_start(kh4[:, :, 128:512], kh4_d[:, :, 128:512])
            nc.sync.dma_start(khi[:, :, 512:1024], khi_d[:, :, 512:1024])
            nc.sync.dma_start(klo[:, :, 512:1024], klo_d[:, :, 512:1024])
            if TERMS == 3:
                nc.sync.dma_start(kh4[:, :, 512:1024], kh4_d[:, :, 512:1024])
            hn = [
                hnpool.tile([128, D], bf16, tag=f"hn{kt}", name=f"t_hn{kt}")
                for kt in range(NTT)
            ]

            ident = cst[:, OFF_ID : OFF_ID + 128]
            gall = cpool.tile([128, NJ], f32, tag="gall", name="t_gall")

            # warm the PE clock ramp while the input DMAs stream
            psw = ps512.tile([128, QT], f32, tag="sc", bufs=4 if RZ_DMA else 2, name="t_warmps")
            for _ in range(12):
                nc.tensor.matmul(
                    psw[:], warm[:, 0:128], warm[:], start=True, stop=True
                )

            def zl_ap(q):
                o = OFF_ZL + 8 * q
                return cst[:, o : o + 8]

            def sel_ap(q):
                o = OFF_SEL + 128 * q
                return cst[0:8, o : o + 128]

            HT2 = TC // 2
            sets = [(khi, ht, NKD // 2), (klo, ht, NKD // 2)]
            if TERMS == 3 and NG3 > 0:
                sets.append((kh4, hlo))
            NPROD = NKD + NG3 if TERMS == 3 else NKD

            st = {}  # per-pair live state

            def score_mm(j, q):
                """fp8 DoubleRow score matmuls for quarter q -> exp."""
                S = st[j]
                ps = ps512.tile([128, QT], f32, tag="sc", bufs=4 if RZ_DMA else 2, name="t_ps512")
                for c in range(2):
                    t0 = QT * q + 256 * c
                    n = 0
                    for se in sets:
                        ng = se[2] if len(se) > 2 else NG3
                        kt_, dt_ = se[0], se[1]
                        for gi in range(ng):
                            g = 2 * gi
                            n += 1
                            nc.tensor.matmul(
                                ps[:, 256 * c : 256 * (c + 1)],
                                kt_[:, g : g + 2, 128 * j : 128 * (j + 1)],
                                dt_[:, g : g + 2, t0 : t0 + 256],
                                start=(n == 1),
                                stop=(n == NPROD),
                                perf_mode=DR,
                            )
                nc.scalar.activation(
                    S["e"][:, QT * q : QT * (q + 1)], ps[:], AF.Exp, scale=1.0 / 64.0
                )

            def zred(j, q):
                S = st[j]
                nc.tensor.matmul(
                    S["pz"][:],
                    zl_ap(q),
                    S["e"][:, QT * q : QT * (q + 1)],
                    start=(q == 0),
                    stop=(q == NQ - 1),
                )

            def a_begin(j):
                st[j] = {
                    "e": epool.tile([128, TC], bf16, tag="e", name="t_e"),
                    "pz": psz.tile([8, QT], f32, tag="psz", name="t_psz"),
                }

            def a_recip(j):
                """alpha/Z (zl holds 1/alpha, so pz = Z/alpha)."""
                S = st[j]
                rz32 = rzpool.tile([8, QT], f32, tag="rz32", name="t_rz32")
                rz = rzpool.tile([8, QT], bf16, tag="rz", name="t_rz")
                nc.vector.reciprocal_approx_fast(out=rz32[:], in_=S["pz"][:])
                nc.scalar.copy(rz[:], rz32[:])
                S["rz"] = rz
                if RZ_DMA:
                    # bounce alpha/Z through DRAM to broadcast across
                    # partitions: row 2q+l of rz -> slot-partitions of head l
                    nc.sync.dma_start(
                        rz8_d[j][:].rearrange("l (q t) -> l q t", q=NQ),
                        rz[:].rearrange("(q l) t -> l q t", l=2),
                    )
                    rzb = rzpool.tile([128, TC], bf16, tag="rzb", name="t_rzb")
                    nc.sync.dma_start(
                        rzb[:].rearrange("(l s) t -> s l t", l=2),
                        rz8_d[j][:].partition_broadcast(64),
                    )
                    S["rzb"] = rzb

            def u_q(j, q):
                """u = e * (alpha/Z broadcast), one quarter (pu matmul + DVE)."""
                S = st[j]
                if q == 0:
                    S["u"] = upool.tile([128, TC], bf16, tag="u", name="t_u")
                if RZ_DMA:
                    if q == 0:
                        nc.vector.tensor_tensor(
                            out=S["u"][:], in0=S["e"][:], in1=S["rzb"][:],
                            op=ALU.mult,
                        )
                    return
                pu = ps512.tile([128, QT], f32, tag="pu", bufs=2, name="t_psu")
                nc.tensor.matmul(pu[:], sel_ap(q), S["rz"][:], start=True, stop=True)
                nc.vector.tensor_tensor(
                    out=S["u"][:, QT * q : QT * (q + 1)],
                    in0=S["e"][:, QT * q : QT * (q + 1)],
                    in1=pu[:],
                    op=ALU.mult,
                )

            def g_h(j, h):
                """g = 1 - u on Pool (SBUF-only), one half."""
                S = st[j]
                if h == 0:
                    S["g"] = work.tile([128, TC], f32, tag="g", name="t_g")
                lo, hi = h * HT2, (h + 1) * HT2
                if j == NJ - 1:
                    nc.scalar.activation(
                        S["g"][:, lo:hi], S["u"][:, lo:hi], AF.Copy,
                        bias=1.0, scale=-1.0,
                    )
                else:
                    nc.gpsimd.tensor_scalar(
                        out=S["g"][:, lo:hi],
                        in0=S["u"][:, lo:hi],
                        scalar1=-1.0,
                        scalar2=1.0,
                        op0=ALU.mult,
                        op1=ALU.add,
                    )

            def scan_h(j, h):
                """suffix scan half (DVE)."""
                S = st[j]
                if h == 0:
                    S["suf"] = work.tile([128, TC + 1], bf16, tag="suf", name="t_suf")
                    nc.vector.memset(S["suf"][:, 0:1], 1.0)
                suf = S["suf"]
                lo, hi = h * HT2, (h + 1) * HT2
                nc.vector.tensor_tensor_scan(
                    out=suf[:, lo + 1 : hi + 1],
                    data0=S["g"][:, lo:hi],
                    data1=S["g"][:, lo:hi],
                    initial=1.0 if h == 0 else suf[:, lo : lo + 1],
                    op0=ALU.mult,
                    op1=ALU.bypass,
                )

            def weff_h(j, h):
                """w_eff = u * suffix_exclusive, one half (DVE)."""
                S = st[j]
                if h == 0:
                    S["w"] = work.tile([128, TC], bf16, tag="weff", name="t_weff")
                lo, hi = h * HT2, (h + 1) * HT2
                nc.vector.tensor_tensor(
                    out=S["w"][:, lo:hi],
                    in0=S["u"][:, lo:hi],
                    in1=S["suf"][:, lo:hi],
                    op=ALU.mult,
                )

            def b1_tp(j):
                """transpose w_eff -> [t, hs] (PE); gr=0 drained later by ACT."""
                S = st[j]
                w_j = S["w"]
                wT_j = wtpool.tile([128, TC], bf16, tag="weffT", name="t_weffT")
                S["pt0"] = None
                for gr in range(2):
                    pt = pstp.tile([128, 1024], bf16, tag="pstp", name="t_pstp")
                    for bb in range(8):
                        kt = 8 * gr + bb
                        nc.tensor.transpose(
                            pt[:, 128 * bb : 128 * (bb + 1)],
                            w_j[:, 128 * kt : 128 * (kt + 1)],
                            ident,
                        )
                    if gr == 0:
                        S["pt0"] = pt
                    elif j >= NJ - 2:
                        nc.vector.tensor_copy(
                            out=wT_j[:, 1024 * gr : 1024 * (gr + 1)], in_=pt[:]
                        )
                    else:
                        nc.scalar.copy(wT_j[:, 1024 * gr : 1024 * (gr + 1)], pt[:])
                nc.vector.tensor_copy(
                    out=gall[:, j : j + 1], in_=S["suf"][:, TC : TC + 1]
                )
                S["wT"] = wT_j

            def b1_drain_act(j):
                S = st[j]
                nc.scalar.copy(S["wT"][:, 0:1024], S["pt0"][:])

            def b2_einsum(j):
                S = st[j]
                wT_j = S["wT"]
                pS = pse.tile([128, 128], f32, tag="pse", name="t_pse")
                for kt in range(NTT):
                    nc.tensor.matmul(
                        pS[:],
                        wT_j[:, 128 * kt : 128 * (kt + 1)],
                        hn[kt][:, 128 * j : 128 * (j + 1)],
                        start=(kt == 0),
                        stop=(kt == NTT - 1),
                    )
                s_sb = wtpool.tile([128, 128], f32, tag="ssb", name="t_ssb")
                nc.scalar.copy(s_sb[:], pS[:])
                nc.sync.dma_start(s_d[j], s_sb[:])
                del st[j]

            # Software pipeline, issue-ordered so no engine queue head blocks
            # on work that is not yet ready; the u->g->scan->w_eff chain of
            # consecutive pairs is interleaved at quarter/half granularity.
            for s in range(NJ + 2):
                A = s < NJ          # score pair s
                U = 1 <= s <= NJ    # u/g pair s-1
                B = 2 <= s <= NJ + 1  # scan/weff/tp pair s-2
                if A:
                    a_begin(s)
                    score_mm(s, 0)
                    score_mm(s, 1)
                    zred(s, 0)
                if U:
                    u_q(s - 1, 0)
                if B:
                    scan_h(s - 2, 0)
                if U:
                    u_q(s - 1, 1)
                if B:
                    weff_h(s - 2, 0)
                if U:
                    g_h(s - 1, 0)
                if A:
                    score_mm(s, 2)
                    zred(s, 1)
                if U:
                    u_q(s - 1, 2)
                if B:
                    scan_h(s - 2, 1)
                if U:
                    u_q(s - 1, 3)
                if B:
                    weff_h(s - 2, 1)
                if U:
                    g_h(s - 1, 1)
                if A:
                    score_mm(s, 3)
                    zred(s, 2)
                if B:
                    b1_tp(s - 2)
                if 4 <= s <= NJ:
                    b2_einsum(s - 4)
                if A:
                    zred(s, 3)
                    a_recip(s)
                if B:
                    b1_drain_act(s - 2)
                if s < NQ:
                    for kt in range(4 * s, 4 * s + 4):
                        nc.sync.dma_start(hn[kt][:], hn_d[:, kt, :])
                if s == NJ + 1:
                    for jj in range(NJ - 3, NJ):
                        b2_einsum(jj)

            nc.sync.dma_start(g_d[:], gall[:])

    nc.compile()
    return nc


def _host_consts():
    cst = np.zeros((128, CST_W), np.float32)
    cst[:, OFF_ID : OFF_ID + 128] = np.eye(128, dtype=np.float32)
    for q in range(NQ):
        for l in range(2):
            o = OFF_ZL + 8 * q
            cst[l * 64 : (l + 1) * 64, o + 2 * q + l] = 1.0 / ALPHA
            o = OFF_SEL + 128 * q
            cst[2 * q + l, o + l * 64 : o + (l + 1) * 64] = 1.0
    return cst


def _chunk128(a):
    """[D, X] -> [128, D//128, X] with d = 128*k + p."""
    Dx, X = a.shape
    return np.ascontiguousarray(a.reshape(Dx // 128, 128, X).transpose(1, 0, 2))


def kernel(H, prototypes, W_tok):
    global _BUILT, LAST_RESULTS
    from concourse.bass_utils import run_bass_kernel_spmd

    if _BUILT is None:
        _BUILT = _build_module()
    nc = _BUILT

    bf = ml_dtypes.bfloat16
    e4 = ml_dtypes.float8_e4m3
    H = np.asarray(H, np.float32)
    prototypes = np.asarray(prototypes, np.float32)
    W_tok = np.asarray(W_tok, np.float32)

    # Keff[(h,s), din] = sum_dh proto[s, h*64+dh] * W[h*64+dh, din] / 8
    P3 = prototypes.reshape(NS, NH, HD)
    W3 = W_tok.reshape(NH, HD, D)
    Keff = (np.einsum("shd,hdn->hsn", P3, W3) / 8.0).reshape(NH * NS, D)

    K_hi = (Keff * 64.0).astype(e4)
    K_res = Keff - K_hi.astype(np.float32) / 64.0
    K_lo = (K_res * 64.0).astype(e4)
    khi_dev = _chunk128(K_hi.T.astype(np.float32)).astype(e4)
    klo_dev = _chunk128(K_lo.T.astype(np.float32)).astype(e4)
    if TERMS == 3:
        kh4_dev = _chunk128((Keff * 4.0).T[: 256 * NG3]).astype(e4)

    cst = _host_consts().astype(bf)

    in_maps = []
    for c in range(NCORES):
        b, th = c // 2, c % 2
        Hs = np.ascontiguousarray(H[b, th * TC : (th + 1) * TC, :][::-1])  # [TC, D]
        H8 = Hs.astype(e4)
        m = {
            "khi": khi_dev,
            "klo": klo_dev,
            "ht": _chunk128(np.ascontiguousarray(H8.astype(np.float32).T)).astype(e4),
            "hn": np.ascontiguousarray(
                Hs.reshape(NTT, 128, D).transpose(1, 0, 2)
            ).astype(bf),
            "cst": cst,
            "warm": np.full((128, 512), 0.25, bf),
        }
        if TERMS == 3:
            Hlo = (Hs - H8.astype(np.float32)) * 16.0
            m["kh4"] = kh4_dev
            m["hlo"] = _chunk128(np.ascontiguousarray(Hlo.T[: 256 * NG3])).astype(e4)
        in_maps.append(m)

    res = run_bass_kernel_spmd(nc, in_maps, core_ids=list(range(NCORES)))
    LAST_RESULTS = res

    percore = []
    for c in range(NCORES):
        r = res.results[c]
        s_blocks = r["s_out"]  # [NJ, 128, 128]
        g_cols = r["g_out"]  # [128, NJ]
        S_c = np.zeros((NH, NS, HD), np.float32)
        G_c = np.zeros((NH, NS), np.float32)
        for j in range(NJ):
            for l in range(2):
                h = 2 * j + l
                S_c[h] = s_blocks[j, l * 64 : (l + 1) * 64, l * 64 : (l + 1) * 64]
                G_c[h] = g_cols[l * 64 : (l + 1) * 64, j]
        percore.append((S_c, G_c))

    out = np.zeros((B, NS, D), np.float32)
    for b in range(B):
        S_e, G_e = percore[2 * b]      # tokens [0, TC)
        S_l, G_l = percore[2 * b + 1]  # tokens [TC, T)
        S_b = S_l + G_l[:, :, None] * S_e
        out[b] = S_b.transpose(1, 0, 2).reshape(NS, D)
    return out
